# revision 30
# baseline (speedup 1.0000x reference)
"""GAT layer kernel for Trainium2, 8 NeuronCores, batch-sharded.

Math (per graph g of B=128, M=512 nodes, in=128, out D=64):
    Wh = h @ W.T;  s_src = Wh @ a[:D];  s_dst = Wh @ a[D:]
    e[i,j] = leakyrelu_0.2(s_src[i] + s_dst[j])
    out = elu(softmax(e, -1) @ Wh)

Device formulation (per core: 16 graphs). All matmuls are bf16: fp32
matmuls on TRN2 run as two hi/lo passes, and per-matmul overhead
(~219 cycles) dominates at these shapes, so matmul COUNT is minimized.
Host folds a into the weight, Wq = [W.T | W.T@a_dst | W.T@a_src], and
pre-transposes h -> hT [16, 128k, 512m] so no on-chip transposes are
needed. Per graph (21 matmuls):
  - 4 matmuls  lhsT=hT[:,c*128:...], rhs=Wq -> psum [n, 66]
    = [Wh | s_dst | s_src]; cast to SBUF bf16 with a ones column
    appended -> WhA [n, 67]; s_dst columns also kept in fp32 (sdcol).
  - 1 matmul   lhsT=wsb (host constant whose every column is w_src):
    E0[n, m] = s_src[m] broadcast directly from hT (psum).
  - leakyrelu(E0 + s_dst): s_dst rides as the per-partition bias;
    chunks split between ACT (Prelu w/ bias+alpha, one op each) and
    DVE (two ops each), alternating 3/1 and 2/2 per graph to balance
    engine load. Then one merged ACT Exp over
    [128, 2048] -> P bf16 (softmax numerator, [n, m] layout). Prelu and
    Exp share the exp_and_others ACT table set -> no table reloads.
    Softmax max-subtraction is skipped: |e| < ~10, exp cannot overflow,
    and softmax is shift-invariant.
  - 16 matmuls lhsT=P[:, c, mc*128:...], rhs=WhA_c accumulating
    psum [m, 67]: cols 0:64 = unnormalized A@Wh, col 66 = softmax
    denominator Z_m (from the ones column).
  - final: r=1/Z (per-partition); elu(x*r) = (relu(x*r) - 1) +
    exp(min(x*r, 0)) on DVE (with a free-axis 0-stride broadcast of r)
    plus one small ACT Exp.
"""

import os
import sys
import types
from contextlib import ExitStack

import numpy as np
import ml_dtypes

# Defensive: concourse.bass_utils imports antenv.axon_hooks when tracing is
# requested (BASS_TRACE). Some images lack that module; register a stub so a
# traced run degrades to untraced instead of crashing.
try:
    import antenv.axon_hooks  # noqa: F401
except Exception:
    try:
        import antenv

        _hooks = types.ModuleType("antenv.axon_hooks")
        _hooks._hook = None
        _hooks.set_axon_ntff_profile_hook = lambda h: setattr(_hooks, "_hook", h)
        _hooks.get_axon_ntff_profile_hook = lambda: _hooks._hook
        sys.modules["antenv.axon_hooks"] = _hooks
        antenv.axon_hooks = _hooks
    except Exception:
        pass

import concourse.bass as bass
import concourse.tile as tile
from concourse import bacc, mybir
from concourse._compat import with_exitstack
from concourse.bass import ds, ts
from concourse.bass_utils import run_bass_kernel_spmd

B, M, IN_DIM, D = 128, 512, 128, 64
N_CORES = 8
G = B // N_CORES  # graphs per core
NC = M // 128  # 128-node chunks per graph
ALPHA = 0.2
F32 = mybir.dt.float32
BF16 = mybir.dt.bfloat16

LAST_RESULTS = None  # BassKernelResults of the most recent run (for test.py)


@with_exitstack
def _gat_body(ctx: ExitStack, tc: tile.TileContext, out_ap, ht_ap, wq_ap, wsb_ap):
    nc = tc.nc
    const = ctx.enter_context(tc.tile_pool(name="const", bufs=1))
    ht_pool = ctx.enter_context(tc.tile_pool(name="ht", bufs=4))
    wha_pool = ctx.enter_context(tc.tile_pool(name="wha", bufs=4))
    row_pool = ctx.enter_context(tc.tile_pool(name="rows", bufs=2))
    p_pool = ctx.enter_context(tc.tile_pool(name="p", bufs=4))
    fin_pool = ctx.enter_context(tc.tile_pool(name="fin", bufs=4))
    out_pool = ctx.enter_context(tc.tile_pool(name="out", bufs=3))
    ps_wh = ctx.enter_context(tc.tile_pool(name="ps_wh", bufs=2, space="PSUM"))
    ps_e = ctx.enter_context(tc.tile_pool(name="ps_e", bufs=3, space="PSUM"))
    ps_o = ctx.enter_context(tc.tile_pool(name="ps_o", bufs=3, space="PSUM"))

    wq_s = const.tile([IN_DIM, D + 2], BF16)
    nc.sync.dma_start(wq_s[:], wq_ap[:])

    # wsb[k, n] = w_src[k] (host constant, all columns identical): one
    # matmul lhsT=wsb, rhs=hT gives E0[n, m] = s_src[m] directly --
    # no s-row matmul, no psum->sbuf row hop.
    wsb_s = const.tile([IN_DIM, 128], BF16)
    nc.sync.dma_start(wsb_s[:], wsb_ap[:])

    # Dummy activation at kernel start: triggers the one-time ~2.7us
    # exp_and_others ACT table load while the first ht DMA is in flight,
    # instead of on graph 0's critical path.
    warm = const.tile([1, 16], F32)
    nc.vector.memset(warm[:], 0.0)
    nc.scalar.activation(warm[:], warm[:], mybir.ActivationFunctionType.Prelu, alpha=ALPHA)

    for g in range(G):
        ht_s = ht_pool.tile([IN_DIM, M], BF16)
        nc.sync.dma_start(ht_s[:], ht_ap[g])

        # Wh + score columns for each node chunk: psum [128n, 66]
        p_wh = ps_wh.tile([128, NC, D + 2], F32)
        for c in range(NC):
            nc.tensor.matmul(
                p_wh[:, c, :], ht_s[:, ts(c, 128)], wq_s[:], start=True, stop=True
            )
        # WhA [n, 67] = [Wh | s_dst | s_src | 1]  (bf16)
        wha = wha_pool.tile([128, NC, D + 3], BF16)
        nc.vector.tensor_copy(wha[:, :, 0 : D + 2], p_wh[:, :, :])
        nc.vector.memset(wha[:, :, D + 2 : D + 3], 1.0)
        # fp32 copy of the s_dst columns (ACT/DVE bias scalars need fp32)
        sdcol = fin_pool.tile([128, NC, 1], F32, tag="sdcol")
        nc.vector.tensor_copy(sdcol[:], p_wh[:, :, D : D + 1])

        # E0[n, m] = s_src[m]: one matmul, every lhsT column is w_src
        p_e0 = ps_e.tile([128, M], F32)
        nc.tensor.matmul(p_e0[:], wsb_s[:], ht_s[:], start=True, stop=True)

        # leakyrelu(s_src[m] + s_dst[n]) per node chunk: s_dst rides as the
        # per-partition bias. Three chunks on ACT (Prelu w/ bias, one op),
        # one chunk on DVE (two ops) to balance the engines. Prelu/Exp
        # share the exp_and_others ACT table set -> no table reloads.
        p1 = p_pool.tile([128, NC, M], BF16, tag="p1")
        n_dve = 1 if g % 2 == 0 else 2  # avg 2.5 ACT / 1.5 DVE chunks
        for c in range(NC - n_dve):
            nc.scalar.activation(
                p1[:, c, :],
                p_e0[:],
                mybir.ActivationFunctionType.Prelu,
                bias=sdcol[:, c, :],
                alpha=ALPHA,
            )
        for c in range(NC - n_dve, NC):
            t1 = p_pool.tile([128, M], BF16, tag="t1")
            nc.vector.tensor_scalar(
                t1[:], p_e0[:], sdcol[:, c, :], ALPHA,
                mybir.AluOpType.add, mybir.AluOpType.mult,
            )
            nc.vector.scalar_tensor_tensor(
                p1[:, c, :], p_e0[:], sdcol[:, c, :], t1[:],
                mybir.AluOpType.add, mybir.AluOpType.max,
            )
        p_t = p_pool.tile([128, NC, M], BF16, tag="p")
        nc.scalar.activation(p_t[:], p1[:], mybir.ActivationFunctionType.Exp)

        # attention: psum [m, 67]; col 66 = Z_m
        p_o = ps_o.tile([128, NC, D + 3], F32)
        for mc in range(NC):
            for c in range(NC):
                nc.tensor.matmul(
                    p_o[:, mc, :],
                    p_t[:, c, ds(mc * 128, 128)],
                    wha[:, c, :],
                    start=(c == 0),
                    stop=(c == NC - 1),
                )

        # final: elu(x*r) = (relu(x*r) - 1) + exp(min(x*r, 0)),  r = 1/Z
        r4 = fin_pool.tile([128, NC], F32)
        nc.vector.reciprocal(r4[:], p_o[:, :, D + 2])
        x_t = fin_pool.tile([128, NC, D], F32)
        r4b = r4[:].unsqueeze(2).broadcast_to([128, NC, D])
        nc.vector.tensor_tensor(
            x_t[:], p_o[:, :, 0:D], r4b, mybir.AluOpType.mult
        )
        # elu(x) = relu(x) - 1 + min(exp(x), 1)  (= relu(x)-1+exp(min(x,0)));
        # saves the min(x,0) DVE pass by letting ACT exp read x directly.
        w_t = fin_pool.tile([128, NC, D], F32)
        nc.vector.tensor_scalar(
            w_t[:], x_t[:], 0.0, 1.0, mybir.AluOpType.max, mybir.AluOpType.subtract
        )
        v_t = fin_pool.tile([128, NC, D], F32)
        nc.scalar.activation(v_t[:], x_t[:], mybir.ActivationFunctionType.Exp)
        m_t = fin_pool.tile([128, NC, D], F32)
        nc.vector.tensor_scalar(
            m_t[:], v_t[:], 1.0, None, mybir.AluOpType.min
        )
        o_t = out_pool.tile([128, NC, D], F32)
        nc.vector.tensor_tensor(o_t[:], m_t[:], w_t[:], mybir.AluOpType.add)
        for c in range(NC):
            nc.sync.dma_start(out_ap[g, ds(c * 128, 128), :], o_t[:, c, :])


_CACHE = {}


def _build():
    if "nc" in _CACHE:
        return _CACHE["nc"]
    nc = bacc.Bacc(
        "TRN2", target_bir_lowering=False, debug=False, num_devices=N_CORES
    )
    ht_d = nc.dram_tensor("ht", [G, IN_DIM, M], BF16, kind="ExternalInput")
    wq_d = nc.dram_tensor("wq", [IN_DIM, D + 2], BF16, kind="ExternalInput")
    wsb_d = nc.dram_tensor("wsb", [IN_DIM, 128], BF16, kind="ExternalInput")
    out_d = nc.dram_tensor("out", [G, M, D], F32, kind="ExternalOutput")
    with tile.TileContext(nc) as tc:
        _gat_body(tc, out_d.ap(), ht_d.ap(), wq_d.ap(), wsb_d.ap())
    nc.compile()
    _CACHE["nc"] = nc
    return nc


def kernel(h, W, a):
    global LAST_RESULTS
    h = np.asarray(h, dtype=np.float32)
    W = np.asarray(W, dtype=np.float32)
    a = np.asarray(a, dtype=np.float32)

    wt = W.T.astype(np.float32)  # [128, 64]
    wq = np.concatenate(
        [wt, (wt @ a[D:])[:, None], (wt @ a[:D])[:, None]], axis=1
    ).astype(ml_dtypes.bfloat16)  # [128, 66] = [W.T | w_dst | w_src]
    w_src = (wt @ a[:D]).astype(ml_dtypes.bfloat16)  # [128]
    wsb = np.ascontiguousarray(np.repeat(w_src[:, None], 128, axis=1))

    nc = _build()
    in_maps = []
    for c in range(N_CORES):
        h_c = h[c * G : (c + 1) * G]  # [G, 512, 128]
        ht_c = np.ascontiguousarray(h_c.transpose(0, 2, 1)).astype(
            ml_dtypes.bfloat16
        )  # [G, 128, 512]
        in_maps.append({"ht": ht_c, "wq": wq, "wsb": wsb})

    res = run_bass_kernel_spmd(nc, in_maps, list(range(N_CORES)))
    LAST_RESULTS = res
    out = np.concatenate([r["out"] for r in res.results], axis=0)
    return out.astype(np.float32)
